# revision 28
# baseline (speedup 1.0000x reference)
"""Trainium2 Bass kernel for the CAM (cross-attention module) problem.

Math (per sample b):
    img = w_img @ x1_b          # [256, 4096]
    kv  = w_txt @ x2_b          # [256, 4096]
    attn = softmax(img @ kv^T)  # [256, 256], softmax over last dim
    y = gamma * (attn @ kv) + img
    out_b = w_out @ y           # [512, 4096]

Sharding: data-parallel over batch, 16 samples -> 2 per core x 8 cores,
no collectives.

Precision: fp16 datapath (fp32 PSUM accumulation), fp16 output stores
(host casts back to fp32; ~2.5e-4 extra rounding, gate is 2e-2).

Layout strategy: all HBM tensors are pre-tiled on the HOST into the
exact [partition, free] SBUF layouts the kernel consumes, so every DMA
moves fully contiguous 2-4KB lines per partition (the natural
channel-major layout only gives 1KB strided lines, which measured
~87 GB/s on the input path vs ~300+ contiguous).  The output is written
tiled+fp16 and un-tiled on the host.

The spatial contraction (attn logits) needs spatial-major operands;
img/kv are computed channel-major and the spatial-major copies are made
with PE transposes (fp16 identity).  PSUM->SBUF drains split DVE/Act.
The two samples per core are software-pipelined as in the baseline.

Startup: the first ~7us are a fixed framework preamble; input DMAs then
need ~3-4us to land.  A short run of dummy matmuls on a memset tile
keeps the PE busy from the end of the preamble so the HAM clock gate
(4/8 throttle when idle) is already released when real data arrives,
and weight DMAs issue on the Scalar HWDGE queue in parallel with the
input DMAs on the Sync queue.
"""

import numpy as np

# Problem shapes (hardcoded per the harness contract)
B = 16
C1 = 512          # x1 channels (also output channels)
C2 = 320          # x2 channels
C2P = 384         # x2 channels padded to a multiple of 128 (K<128 matmuls
                  # and partition-offset memsets are both broken on HW)
C = 256           # projected channels
HW = 64 * 64      # spatial size
NCORES = 8
SPC = B // NCORES  # samples per core

_BUILD_CACHE = {}

N_WARMUP = 38      # dummy PE matmuls covering the input-DMA head


def _nonce_len():
    import inspect
    import zlib
    return 2 + (zlib.crc32(inspect.getsource(_build_nc).encode()) % 997)


def _build_nc(spc=SPC, c1=C1, c2p=C2P, c=C, hw=HW, ch=512):
    """Build the per-core Bass program (same program on all cores)."""
    import concourse.tile as tile
    from concourse import bacc, mybir

    f32 = mybir.dt.float32
    f16 = mybir.dt.float16
    P = 128
    K1 = c1 // P           # k-tiles of x1 channels
    K2 = c2p // P          # k-tiles of x2 channels (padded)
    KC = c // P            # k-tiles of projected channels
    MO = c1 // P           # m-tiles of output conv
    NCH = hw // ch         # spatial chunks
    TPC = ch // P          # 128-wide spatial tiles per chunk

    # Bacc (not plain Bass): its compile() runs move_matmul_waits_to_ldweights
    # + generate_event_semaphores, without which walrus rejects any Matmult
    # carrying more than one semaphore wait.
    nc = bacc.Bacc("TRN2", target_bir_lowering=False)
    # Host-pre-tiled layouts: every tensor is already [partition, free...]
    x1 = nc.declare_dram_parameter("x1", [spc, NCH, P, K1, ch], f16, isOutput=False)
    x2 = nc.declare_dram_parameter("x2", [spc, NCH, P, K2, ch], f16, isOutput=False)
    wiT = nc.declare_dram_parameter("w_imgT", [P, K1, c], f16, isOutput=False)
    wtT = nc.declare_dram_parameter("w_txtT", [P, K2, c], f16, isOutput=False)
    woT = nc.declare_dram_parameter("w_outT", [P, KC, c1], f16, isOutput=False)
    gamma = nc.declare_dram_parameter("gamma", [1], f32, isOutput=False)
    idin = nc.declare_dram_parameter("ident", [P, P], f16, isOutput=False)
    # The PJRT executable cache fingerprints the HLO without the embedded
    # BIR payload, so two different kernels with identical I/O signatures
    # collide. A source-hash-sized dummy input makes the signature unique.
    nc.declare_dram_parameter("nonce", [1, _nonce_len()], f32, isOutput=False)
    out = nc.declare_dram_parameter("out", [spc, NCH, P, MO, ch], f16, isOutput=True)

    Exp = mybir.ActivationFunctionType.Exp
    X = mybir.AxisListType.X

    with (
        tile.TileContext(nc) as tc,
        tc.tile_pool(name="singles", bufs=1) as singles,
        tc.tile_pool(name="xin", bufs=2) as xin,
        tc.tile_pool(name="tch", bufs=2) as tch,
        tc.tile_pool(name="imgp", bufs=2) as imgp,
        tc.tile_pool(name="kvp", bufs=NCH + 2) as kvp,
        tc.tile_pool(name="attnsb", bufs=2) as attnsb,
        tc.tile_pool(name="smalls", bufs=4) as smalls,
        tc.tile_pool(name="ostage", bufs=3) as ostage,
        tc.tile_pool(name="psA", bufs=4, space="PSUM") as psA,
        tc.tile_pool(name="psB", bufs=2, space="PSUM") as psB,
        tc.tile_pool(name="psAttn", bufs=2, space="PSUM") as psAttn,
    ):
        # ---- constants.  Weights go out on the Scalar HWDGE queue so
        # their descriptor generation overlaps the Sync-queue input loads;
        # woT/ident/gamma are deferred past the first chunk.
        wiT_sb = singles.tile([P, K1, c], f16)
        wtT_sb = singles.tile([P, K2, c], f16)
        woT_sb = singles.tile([P, KC, c1], f16)
        ident = singles.tile([P, P], f16)
        gamma_sb = singles.tile([P, 1], f32)
        warm = singles.tile([P, P], f16)

        def emit_warmup():
            # Keep the PE streaming from the end of the framework preamble
            # until the first input chunk lands: releases the HAM clock
            # throttle (~3.4us of sustained activity) before real matmuls.
            nc.gpsimd.memset(warm[:, :], 0)
            for i in range(N_WARMUP):
                ps = psA.tile([P, ch], f32, tag="a", name="ps_img")
                nc.tensor.matmul(ps[:, 0:P], lhsT=warm, rhs=warm,
                                 start=True, stop=True)

        def emit_deferred_constants():
            nc.scalar.dma_start(out=woT_sb[:, :, :], in_=woT[:, :, :])
            nc.scalar.dma_start(out=ident, in_=idin[:])
            nc.scalar.dma_start(out=gamma_sb, in_=gamma[:].to_broadcast((P, 1)))

        # ---- per-sample emission helpers -------------------------------
        # DMA instruction issue costs ~600ns of HWDGE sequencer time each,
        # so inputs are loaded with ONE contiguous DMA per chunk.
        def passA_chunk(st, s, cc, first=False):
            cs = cc * ch
            if first:
                # wiT + the two x1 halves go sequentially on the sync queue
                # (it starts moving ~2us before the scalar queue).  The two
                # halves are SEPARATE tiles: Tile tracks dependencies per
                # tile, so the k-interleaved first matmuls really do start
                # after only the k01 half lands.
                x1a = xin.tile([P, 2, ch], f16, tag="x1a", name="x1a")
                x1b = xin.tile([P, 2, ch], f16, tag="x1b", name="x1b")
                nc.sync.dma_start(out=wiT_sb[:, :, :], in_=wiT[:, :, :])
                nc.sync.dma_start(out=x1a[:, :, :], in_=x1[s, cc, :, 0:2, :])
                nc.sync.dma_start(out=x1b[:, :, :], in_=x1[s, cc, :, 2:4, :])
                nc.scalar.dma_start(out=wtT_sb[:, :, :], in_=wtT[:, :, :])
                x1k = lambda k: (x1a if k < 2 else x1b)[:, k % 2, :]
            else:
                x1c = xin.tile([P, K1, ch], f16, tag="x1c", name="x1c", bufs=3)
                nc.sync.dma_start(out=x1c[:, :, :], in_=x1[s, cc])
                x1k = lambda k: x1c[:, k, :]
            # x2 rides the scalar HWDGE queue: inputs come down two parallel
            # rings, which matters in the early chunks where the PE has no
            # tail work to absorb input-delivery jitter
            x2c = xin.tile([P, K2, ch], f16, tag="x2c", name="x2c", bufs=3)
            nc.scalar.dma_start(out=x2c[:, :, :], in_=x2[s, cc])
            # psum->sbuf drains split DVE/Act so neither copy engine paces PE
            img_ps = [psA.tile([P, ch], f32, tag="a", name="ps_img")
                      for m in range(KC)]
            if first:
                # k-major interleave: the first 4 matmuls need only x1a
                for k in range(K1):
                    for m in range(KC):
                        nc.tensor.matmul(img_ps[m],
                                         lhsT=wiT_sb[:, k, m * P:(m + 1) * P],
                                         rhs=x1k(k),
                                         start=(k == 0), stop=(k == K1 - 1))
            else:
                for m in range(KC):
                    for k in range(K1):
                        nc.tensor.matmul(img_ps[m],
                                         lhsT=wiT_sb[:, k, m * P:(m + 1) * P],
                                         rhs=x1k(k),
                                         start=(k == 0), stop=(k == K1 - 1))
            for m in range(KC):
                if m % 2 == 0:
                    nc.vector.tensor_copy(out=st["img"][:, m, cs:cs + ch],
                                          in_=img_ps[m])
                else:
                    nc.scalar.copy(out=st["img"][:, m, cs:cs + ch],
                                   in_=img_ps[m])
            kvt = kvp.tile([P, KC, ch], f16, tag="kv", name="kvt")
            st["kvch"][cc] = kvt
            for m in range(KC):
                ps = psA.tile([P, ch], f32, tag="a", name="ps_kv")
                for k in range(K2):
                    nc.tensor.matmul(ps, lhsT=wtT_sb[:, k, m * P:(m + 1) * P],
                                     rhs=x2c[:, k, :],
                                     start=(k == 0), stop=(k == K2 - 1))
                if m % 2 == 0:
                    nc.scalar.copy(out=kvt[:, m, :], in_=ps)
                else:
                    nc.vector.tensor_copy(out=kvt[:, m, :], in_=ps)

        def transposes(st, s, pc, inter=()):
            # spatial-major orientations via PE transpose of img / kv chunks.
            # (The DMA xbar transpose was measured: it moves data in ~378B
            # packets, ~3x under line rate, starving the input queue - PE
            # transposes with an fp16 identity are the fast option here.)
            # 4 transposes (2 t-groups) share one psB tile and drain with a
            # single [128,512] DVE copy.  `inter` callables (the previous
            # chunk's attn matmul halves) are emitted between groups so the
            # psB double-buffer ping-pong never stalls the PE: the drain of
            # group g lands while group g+1 plus ~440ns of attn matmuls run.
            pcs = pc * ch
            imgT_c = tch.tile([P, TPC, c], f16, tag="imgT", name="imgT_c")
            txtT_c = tch.tile([P, TPC, c], f16, tag="txtT", name="txtT_c")
            inter = list(inter)
            g = 0
            for tp in range(TPC // 2):
                for which in range(2):
                    ps = psB.tile([P, 2, c], f16, tag="b", name="ps_T")
                    for tt in range(2):
                        t = tp * 2 + tt
                        for i in range(KC):
                            if which == 0:
                                nc.tensor.transpose(
                                    ps[:, tt, i * P:(i + 1) * P],
                                    st["img"][:, i,
                                              pcs + t * P:pcs + (t + 1) * P],
                                    ident)
                            else:
                                nc.tensor.transpose(
                                    ps[:, tt, i * P:(i + 1) * P],
                                    st["kvch"][pc][:, i, t * P:(t + 1) * P],
                                    ident)
                    dst = imgT_c if which == 0 else txtT_c
                    nc.vector.tensor_copy(out=dst[:, tp * 2:tp * 2 + 2, :],
                                          in_=ps)
                    g += 1
                    if g == 2 and inter:
                        inter.pop(0)()
            while inter:
                inter.pop(0)()
            st["imgT"][pc] = imgT_c
            st["txtT"][pc] = txtT_c

        def attn_half(st, s, pc, m):
            if st["attn_ps"] is None:
                st["attn_ps"] = [
                    psAttn.tile([P, c], f32, tag="attn", name=f"attn{s}_{mm}")
                    for mm in range(KC)
                ]
            for t in range(TPC):
                nc.tensor.matmul(
                    st["attn_ps"][m],
                    lhsT=st["imgT"][pc][:, t, m * P:(m + 1) * P],
                    rhs=st["txtT"][pc][:, t, :],
                    start=(pc == 0 and t == 0),
                    stop=(pc == NCH - 1 and t == TPC - 1))
            if m == KC - 1:
                st["imgT"][pc] = st["txtT"][pc] = None

        def attn_chunk(st, s, pc):
            for m in range(KC):
                attn_half(st, s, pc, m)

        def softmax(st, s, cover=()):
            # softmax over the free (d) axis, gamma folded in; transpose to
            # attnT [d, c] for the attn@kv contraction.  `cover` closures are
            # emitted between the DVE/ACT stats and the PE transposes so the
            # tensor engine has work while the serial softmax chain runs.
            attnT_sb = attnsb.tile([P, KC, c], f16, tag="attnT", name="attnT")
            st["attnT"] = attnT_sb
            exps = []
            for m in range(KC):
                nmax = smalls.tile([P, 1], f32, tag="nmax", name="nmax")
                nc.vector.reduce_max(out=nmax, in_=st["attn_ps"][m], axis=X,
                                     negate=True)
                exp_sb = smalls.tile([P, c], f16, tag="exp", name="exp_sb")
                rsum = smalls.tile([P, 1], f32, tag="rsum", name="rsum")
                nc.scalar.activation(out=exp_sb, in_=st["attn_ps"][m], func=Exp,
                                     bias=nmax, scale=1.0, accum_out=rsum)
                rg = smalls.tile([P, 1], f32, tag="rg", name="rg")
                nc.vector.reciprocal(out=rg, in_=rsum)
                nc.vector.tensor_mul(out=rg, in0=rg, in1=gamma_sb)
                nc.vector.tensor_scalar_mul(out=exp_sb, in0=exp_sb, scalar1=rg)
                exps.append(exp_sb)
            for fn in cover:
                fn()
            for m in range(KC):
                for j in range(KC):
                    pst = psB.tile([P, P], f16, tag="b", name="ps_tr")
                    nc.tensor.transpose(pst, exps[m][:, j * P:(j + 1) * P], ident)
                    nc.vector.tensor_copy(out=attnT_sb[:, j, m * P:(m + 1) * P],
                                          in_=pst)

        def ph4_chunk(st, s, cc):
            # y = gamma*attn@kv + img, overwriting img in place
            cs = cc * ch
            for m in range(KC):
                ps = psA.tile([P, ch], f32, tag="a", name="ps_ai")
                for j in range(KC):
                    nc.tensor.matmul(ps, lhsT=st["attnT"][:, j, m * P:(m + 1) * P],
                                     rhs=st["kvch"][cc][:, j, :],
                                     start=(j == 0), stop=(j == KC - 1))
                nc.vector.tensor_add(out=st["img"][:, m, cs:cs + ch], in0=ps,
                                     in1=st["img"][:, m, cs:cs + ch])
            st["kvch"][cc] = None

        def ph5_chunk(st, s, cc):
            # all four m-tiles share one fp16 staging tile and go out with
            # a single contiguous DMA on the Pool engine's software DGE,
            # keeping the HWDGE rings free for input loads.
            cs = cc * ch
            last = (s == spc - 1 and cc == NCH - 1)
            ot = ostage.tile([P, MO, ch], f16, tag="ot", name="ot")
            for m2 in range(MO):
                ps = psA.tile([P, ch], f32, tag="a", name="ps_out")
                for j in range(KC):
                    nc.tensor.matmul(ps,
                                     lhsT=woT_sb[:, j, m2 * P:(m2 + 1) * P],
                                     rhs=st["img"][:, j, cs:cs + ch],
                                     start=(j == 0), stop=(j == KC - 1))
                if m2 % 2 == 0:
                    nc.vector.tensor_copy(out=ot[:, m2, :], in_=ps)
                else:
                    nc.scalar.copy(out=ot[:, m2, :], in_=ps)
                if last and m2 == 1:
                    # stream the final chunk out in pieces across both HWDGE
                    # rings so the kernel end never waits on one big transfer
                    nc.sync.dma_start(out=out[s, cc, :, 0:2, :],
                                      in_=ot[:, 0:2, :])
                if last and m2 == 2:
                    nc.scalar.dma_start(out=out[s, cc, :, 2:3, :],
                                        in_=ot[:, 2:3, :])
            if last:
                nc.sync.dma_start(out=out[s, cc, :, 3:4, :], in_=ot[:, 3:4, :])
            elif s == spc - 1 and cc == NCH - 2:
                # second-to-last store also avoids the backlogged SWDGE ring
                nc.sync.dma_start(out=out[s, cc], in_=ot[:, :, :])
            else:
                nc.gpsimd.dma_start(out=out[s, cc], in_=ot[:, :, :])

        # ---- pipelined schedule: sample s-1's tail (last transposes, attn,
        # softmax, phases 4/5) is interleaved into sample s's pass-A chunks
        # so the PE never drains at sample boundaries.  ph4/ph5 chunks are
        # interleaved pairwise so ph4's DVE residual adds run under ph5's
        # PE-heavy output conv.
        def tail_units(st, s):
            units = []
            order = []
            order.append(("ph4", 0))
            order.append(("ph4", 1))
            for cc in range(2, NCH):
                order.append(("ph5", cc - 2))
                order.append(("ph4", cc))
            order.append(("ph5", NCH - 2))
            order.append(("ph5", NCH - 1))
            for kind, cc in order:
                if kind == "ph4":
                    units.append(lambda st=st, s=s, cc=cc: ph4_chunk(st, s, cc))
                else:
                    units.append(lambda st=st, s=s, cc=cc: ph5_chunk(st, s, cc))
            return units

        emit_warmup()
        tails = []
        for s in range(spc):
            st = {"img": None, "kvch": [None] * NCH, "attn_ps": None,
                  "attnT": None, "imgT": [None] * NCH, "txtT": [None] * NCH}
            st["img"] = imgp.tile([P, KC, hw], f16, tag="img", name=f"img{s}")
            for cc in range(NCH):
                passA_chunk(st, s, cc, first=(s == 0 and cc == 0))
                if s == 0 and cc == 0:
                    emit_deferred_constants()
                if cc >= 1:
                    inter = ()
                    if cc >= 2:
                        inter = (lambda st=st, s=s, cc=cc: attn_half(st, s, cc - 2, 0),
                                 lambda st=st, s=s, cc=cc: attn_half(st, s, cc - 2, 1))
                    transposes(st, s, cc - 1, inter)
                # the last sample keeps one extra ph5 unit in reserve so its
                # softmax (which has no later pass-A to hide behind) gets
                # ~3.5us of covering PE work instead of ~1.7us
                if s == spc - 1:
                    npop = (3, 3, 3, 3, 2, 2, 1, 0)[min(cc, 7)]
                else:
                    npop = (3, 3, 3, 3, 2, 2, 1, 1)[min(cc, 7)]
                for _ in range(npop):
                    if tails:
                        tails.pop(0)()
            if s == spc - 1:
                # the final sample's softmax has no later pass-A to hide
                # behind; cover it with whatever of the previous sample's
                # tail is still pending (its last output-conv chunks).
                leftovers = tails[:]
                tails.clear()
                tails.extend([
                    (lambda st=st, s=s: transposes(
                        st, s, NCH - 1,
                        (lambda: attn_half(st, s, NCH - 2, 0),
                         lambda: attn_half(st, s, NCH - 2, 1)))),
                    (lambda st=st, s=s: attn_chunk(st, s, NCH - 1)),
                    (lambda st=st, s=s, cov=tuple(leftovers):
                        softmax(st, s, cover=cov)),
                ])
            else:
                tails.extend([
                    (lambda st=st, s=s: transposes(
                        st, s, NCH - 1,
                        (lambda: attn_half(st, s, NCH - 2, 0),
                         lambda: attn_half(st, s, NCH - 2, 1)))),
                    (lambda st=st, s=s: attn_chunk(st, s, NCH - 1)),
                    (lambda st=st, s=s: softmax(st, s)),
                ])
            tails.extend(tail_units(st, s))
        while tails:
            tails.pop(0)()

    nc.compile()
    return nc


def _get_nc():
    key = "full"
    if key not in _BUILD_CACHE:
        _BUILD_CACHE[key] = _build_nc()
    return _BUILD_CACHE[key]


LAST_RESULTS = None  # BassKernelResults of the most recent kernel() call

P = 128
K1 = C1 // P
K2 = C2P // P
KC = C // P
MO = C1 // P
CH = 512
NCH = HW // CH


def _tile_input(x, kt):
    """[B, kt*128, HW] -> [B, NCH, 128, kt, CH] contiguous fp16."""
    b = x.shape[0]
    t = x.reshape(b, kt, P, NCH, CH).transpose(0, 3, 2, 1, 4)
    return np.ascontiguousarray(t)


def _tile_weightT(wT, kt, cols):
    """[kt*128, cols] -> [128, kt, cols] contiguous fp16."""
    return np.ascontiguousarray(
        wT.reshape(kt, P, cols).transpose(1, 0, 2).astype(np.float16))


def kernel(x1, x2, w_img, w_txt, w_out, gamma):
    import os
    from concourse.bass_utils import run_bass_kernel_spmd

    x1 = np.asarray(x1, dtype=np.float32).reshape(B, C1, HW).astype(np.float16)
    x2 = np.asarray(x2, dtype=np.float32).reshape(B, C2, HW).astype(np.float16)
    w_img = np.asarray(w_img, dtype=np.float32)
    w_txt = np.asarray(w_txt, dtype=np.float32)
    w_out = np.asarray(w_out, dtype=np.float32)
    gamma = np.ascontiguousarray(np.asarray(gamma, dtype=np.float32)).reshape(1)

    # pad x2 channels 320 -> 384 with zeros so every k-tile is 128 deep
    x2p = np.zeros((B, C2P, HW), dtype=np.float16)
    x2p[:, :C2, :] = x2

    x1t = _tile_input(x1, K1)                     # [B, NCH, P, K1, CH]
    x2t = _tile_input(x2p, K2)                    # [B, NCH, P, K2, CH]

    w_imgT = _tile_weightT(w_img.T, K1, C)        # [P, K1, 256]
    w_txtTf = np.zeros((C2P, C), dtype=np.float16)
    w_txtTf[:C2, :] = w_txt.T.astype(np.float16)
    w_txtT = _tile_weightT(w_txtTf, K2, C)        # [P, K2, 256]
    w_outT = _tile_weightT(w_out.T, KC, C1)       # [P, KC, 512]

    nc = _get_nc()
    ident = np.eye(128, dtype=np.float16)
    in_maps = []
    for core in range(NCORES):
        s0 = core * SPC
        in_maps.append({
            "x1": np.ascontiguousarray(x1t[s0:s0 + SPC]),
            "x2": np.ascontiguousarray(x2t[s0:s0 + SPC]),
            "w_imgT": w_imgT,
            "w_txtT": w_txtT,
            "w_outT": w_outT,
            "gamma": gamma,
            "ident": ident,
            "nonce": np.zeros((1, _nonce_len()), dtype=np.float32),
        })

    kwargs = {}
    if os.environ.get("KERNEL_TRACE"):
        kwargs["trace"] = True
        if os.environ.get("KERNEL_TRACE_DIR"):
            kwargs["tmpdir"] = os.environ["KERNEL_TRACE_DIR"]
    res = run_bass_kernel_spmd(nc, in_maps, core_ids=list(range(NCORES)), **kwargs)
    global LAST_RESULTS
    LAST_RESULTS = res
    outs = [res.results[c]["out"] for c in range(NCORES)]
    full_t = np.concatenate(outs, axis=0)          # [B, NCH, P, MO, CH] f16
    # un-tile: out[s, m*128+p, cc*512+j] = full_t[s, cc, p, m, j]
    full = full_t.transpose(0, 3, 2, 1, 4).reshape(B, C1, HW)
    return full.astype(np.float32).reshape(B, C1, 64, 64)


if __name__ == "__main__":
    rng = np.random.default_rng(0)
    inputs = {
        "x1": rng.standard_normal((B, C1, 64, 64), dtype=np.float32),
        "x2": rng.standard_normal((B, C2, 64, 64), dtype=np.float32),
        "w_img": rng.standard_normal((C, C1), dtype=np.float32) / np.sqrt(C1),
        "w_txt": rng.standard_normal((C, C2), dtype=np.float32) / np.sqrt(C2),
        "w_out": rng.standard_normal((C1, C), dtype=np.float32) / np.sqrt(C),
        "gamma": rng.standard_normal(1).astype(np.float32),
    }
    out = kernel(**inputs)
    print(out.shape, out.dtype)


# revision 30
# speedup vs baseline: 1.0110x; 1.0110x over previous
"""Trainium2 Bass kernel for the CAM (cross-attention module) problem.

Math (per sample b):
    img = w_img @ x1_b          # [256, 4096]
    kv  = w_txt @ x2_b          # [256, 4096]
    attn = softmax(img @ kv^T)  # [256, 256], softmax over last dim
    y = gamma * (attn @ kv) + img
    out_b = w_out @ y           # [512, 4096]

Sharding: data-parallel over batch, 16 samples -> 2 per core x 8 cores,
no collectives.

Precision: fp16 datapath (fp32 PSUM accumulation), fp16 output stores
(host casts back to fp32; ~2.5e-4 extra rounding, gate is 2e-2).

Layout strategy: all HBM tensors are pre-tiled on the HOST into the
exact [partition, free] SBUF layouts the kernel consumes, so every DMA
moves fully contiguous 2-4KB lines per partition (the natural
channel-major layout only gives 1KB strided lines, which measured
~87 GB/s on the input path vs ~300+ contiguous).  The output is written
tiled+fp16 and un-tiled on the host.

The spatial contraction (attn logits) needs spatial-major operands;
img/kv are computed channel-major and the spatial-major copies are made
with PE transposes (fp16 identity).  PSUM->SBUF drains split DVE/Act.
The two samples per core are software-pipelined as in the baseline.

Startup: the first ~7us are a fixed framework preamble; input DMAs then
need ~3-4us to land.  A short run of dummy matmuls on a memset tile
keeps the PE busy from the end of the preamble so the HAM clock gate
(4/8 throttle when idle) is already released when real data arrives,
and weight DMAs issue on the Scalar HWDGE queue in parallel with the
input DMAs on the Sync queue.
"""

import numpy as np

# Problem shapes (hardcoded per the harness contract)
B = 16
C1 = 512          # x1 channels (also output channels)
C2 = 320          # x2 channels
C2P = 384         # x2 channels padded to a multiple of 128 (K<128 matmuls
                  # and partition-offset memsets are both broken on HW)
C = 256           # projected channels
HW = 64 * 64      # spatial size
NCORES = 8
SPC = B // NCORES  # samples per core

_BUILD_CACHE = {}

N_WARMUP = 38      # dummy PE matmuls covering the input-DMA head


def _nonce_len():
    import inspect
    import zlib
    return 2 + (zlib.crc32(inspect.getsource(_build_nc).encode()) % 997)


def _build_nc(spc=SPC, c1=C1, c2p=C2P, c=C, hw=HW, ch=512):
    """Build the per-core Bass program (same program on all cores)."""
    import concourse.tile as tile
    from concourse import bacc, mybir

    f32 = mybir.dt.float32
    f16 = mybir.dt.float16
    P = 128
    K1 = c1 // P           # k-tiles of x1 channels
    K2 = c2p // P          # k-tiles of x2 channels (padded)
    KC = c // P            # k-tiles of projected channels
    MO = c1 // P           # m-tiles of output conv
    NCH = hw // ch         # spatial chunks
    TPC = ch // P          # 128-wide spatial tiles per chunk

    # Bacc (not plain Bass): its compile() runs move_matmul_waits_to_ldweights
    # + generate_event_semaphores, without which walrus rejects any Matmult
    # carrying more than one semaphore wait.
    nc = bacc.Bacc("TRN2", target_bir_lowering=False)
    # Host-pre-tiled layouts: every tensor is already [partition, free...]
    x1 = nc.declare_dram_parameter("x1", [spc, NCH, P, K1, ch], f16, isOutput=False)
    x2 = nc.declare_dram_parameter("x2", [spc, NCH, P, K2, ch], f16, isOutput=False)
    wiT = nc.declare_dram_parameter("w_imgT", [P, K1, c], f16, isOutput=False)
    wtT = nc.declare_dram_parameter("w_txtT", [P, K2, c], f16, isOutput=False)
    woT = nc.declare_dram_parameter("w_outT", [P, KC, c1], f16, isOutput=False)
    gamma = nc.declare_dram_parameter("gamma", [1], f32, isOutput=False)
    idin = nc.declare_dram_parameter("ident", [P, P], f16, isOutput=False)
    # The PJRT executable cache fingerprints the HLO without the embedded
    # BIR payload, so two different kernels with identical I/O signatures
    # collide. A source-hash-sized dummy input makes the signature unique.
    nc.declare_dram_parameter("nonce", [1, _nonce_len()], f32, isOutput=False)
    out = nc.declare_dram_parameter("out", [spc, NCH, P, MO, ch], f16, isOutput=True)

    Exp = mybir.ActivationFunctionType.Exp
    X = mybir.AxisListType.X

    with (
        tile.TileContext(nc) as tc,
        tc.tile_pool(name="singles", bufs=1) as singles,
        tc.tile_pool(name="xin", bufs=2) as xin,
        tc.tile_pool(name="tch", bufs=2) as tch,
        tc.tile_pool(name="imgp", bufs=2) as imgp,
        tc.tile_pool(name="kvp", bufs=NCH + 2) as kvp,
        tc.tile_pool(name="attnsb", bufs=2) as attnsb,
        tc.tile_pool(name="smalls", bufs=4) as smalls,
        tc.tile_pool(name="ostage", bufs=3) as ostage,
        tc.tile_pool(name="psA", bufs=4, space="PSUM") as psA,
        tc.tile_pool(name="psB", bufs=2, space="PSUM") as psB,
        tc.tile_pool(name="psAttn", bufs=2, space="PSUM") as psAttn,
    ):
        # ---- constants.  Weights go out on the Scalar HWDGE queue so
        # their descriptor generation overlaps the Sync-queue input loads;
        # woT/ident/gamma are deferred past the first chunk.
        wiT_sb = singles.tile([P, K1, c], f16)
        wtT_sb = singles.tile([P, K2, c], f16)
        woT_sb = singles.tile([P, KC, c1], f16)
        ident = singles.tile([P, P], f16)
        gamma_sb = singles.tile([P, 1], f32)
        warm = singles.tile([P, P], f16)

        def emit_warmup():
            # Keep the PE streaming from the end of the framework preamble
            # until the first input chunk lands: releases the HAM clock
            # throttle (~3.4us of sustained activity) before real matmuls.
            nc.gpsimd.memset(warm[:, :], 0)
            for i in range(N_WARMUP):
                ps = psA.tile([P, ch], f32, tag="a", name="ps_img")
                nc.tensor.matmul(ps[:, 0:P], lhsT=warm, rhs=warm,
                                 start=True, stop=True)

        def emit_ident():
            nc.scalar.dma_start(out=ident, in_=idin[:])

        def emit_deferred_constants():
            nc.scalar.dma_start(out=woT_sb[:, :, :], in_=woT[:, :, :])
            nc.scalar.dma_start(out=gamma_sb, in_=gamma[:].to_broadcast((P, 1)))

        # ---- per-sample emission helpers -------------------------------
        # DMA instruction issue costs ~600ns of HWDGE sequencer time each,
        # so inputs are loaded with ONE contiguous DMA per chunk.
        def passA_chunk(st, s, cc, first=False):
            cs = cc * ch
            if first:
                # wiT + the two x1 halves go sequentially on the sync queue
                # (it starts moving ~2us before the scalar queue).  The two
                # halves are SEPARATE tiles: Tile tracks dependencies per
                # tile, so the k-interleaved first matmuls really do start
                # after only the k01 half lands.
                x1a = xin.tile([P, 2, ch], f16, tag="x1a", name="x1a")
                x1b = xin.tile([P, 2, ch], f16, tag="x1b", name="x1b")
                nc.sync.dma_start(out=wiT_sb[:, :, :], in_=wiT[:, :, :])
                nc.sync.dma_start(out=x1a[:, :, :], in_=x1[s, cc, :, 0:2, :])
                nc.sync.dma_start(out=x1b[:, :, :], in_=x1[s, cc, :, 2:4, :])
                nc.scalar.dma_start(out=wtT_sb[:, :, :], in_=wtT[:, :, :])
                x1k = lambda k: (x1a if k < 2 else x1b)[:, k % 2, :]
            else:
                x1c = xin.tile([P, K1, ch], f16, tag="x1c", name="x1c", bufs=3)
                nc.sync.dma_start(out=x1c[:, :, :], in_=x1[s, cc])
                x1k = lambda k: x1c[:, k, :]
            # x2 rides the scalar HWDGE queue: inputs come down two parallel
            # rings, which matters in the early chunks where the PE has no
            # tail work to absorb input-delivery jitter
            x2c = xin.tile([P, K2, ch], f16, tag="x2c", name="x2c", bufs=3)
            nc.scalar.dma_start(out=x2c[:, :, :], in_=x2[s, cc])
            # psum->sbuf drains split DVE/Act so neither copy engine paces PE
            img_ps = [psA.tile([P, ch], f32, tag="a", name="ps_img")
                      for m in range(KC)]
            if first:
                # k-major interleave: the first 4 matmuls need only x1a
                for k in range(K1):
                    for m in range(KC):
                        nc.tensor.matmul(img_ps[m],
                                         lhsT=wiT_sb[:, k, m * P:(m + 1) * P],
                                         rhs=x1k(k),
                                         start=(k == 0), stop=(k == K1 - 1))
            else:
                for m in range(KC):
                    for k in range(K1):
                        nc.tensor.matmul(img_ps[m],
                                         lhsT=wiT_sb[:, k, m * P:(m + 1) * P],
                                         rhs=x1k(k),
                                         start=(k == 0), stop=(k == K1 - 1))
            for m in range(KC):
                if m % 2 == 0:
                    nc.vector.tensor_copy(out=st["img"][:, m, cs:cs + ch],
                                          in_=img_ps[m])
                else:
                    nc.scalar.copy(out=st["img"][:, m, cs:cs + ch],
                                   in_=img_ps[m])
            kvt = kvp.tile([P, KC, ch], f16, tag="kv", name="kvt")
            st["kvch"][cc] = kvt
            for m in range(KC):
                ps = psA.tile([P, ch], f32, tag="a", name="ps_kv")
                for k in range(K2):
                    nc.tensor.matmul(ps, lhsT=wtT_sb[:, k, m * P:(m + 1) * P],
                                     rhs=x2c[:, k, :],
                                     start=(k == 0), stop=(k == K2 - 1))
                if m % 2 == 0:
                    nc.scalar.copy(out=kvt[:, m, :], in_=ps)
                else:
                    nc.vector.tensor_copy(out=kvt[:, m, :], in_=ps)

        def transposes(st, s, pc, inter=()):
            # spatial-major orientations via PE transpose of img / kv chunks.
            # (The DMA xbar transpose was measured: it moves data in ~378B
            # packets, ~3x under line rate, starving the input queue - PE
            # transposes with an fp16 identity are the fast option here.)
            # 4 transposes (2 t-groups) share one psB tile and drain with a
            # single [128,512] DVE copy.  `inter` callables (the previous
            # chunk's attn matmul halves) are emitted between groups so the
            # psB double-buffer ping-pong never stalls the PE: the drain of
            # group g lands while group g+1 plus ~440ns of attn matmuls run.
            pcs = pc * ch
            imgT_c = tch.tile([P, TPC, c], f16, tag="imgT", name="imgT_c")
            txtT_c = tch.tile([P, TPC, c], f16, tag="txtT", name="txtT_c")
            inter = list(inter)
            g = 0
            for tp in range(TPC // 2):
                for which in range(2):
                    ps = psB.tile([P, 2, c], f16, tag="b", name="ps_T")
                    for tt in range(2):
                        t = tp * 2 + tt
                        for i in range(KC):
                            if which == 0:
                                nc.tensor.transpose(
                                    ps[:, tt, i * P:(i + 1) * P],
                                    st["img"][:, i,
                                              pcs + t * P:pcs + (t + 1) * P],
                                    ident)
                            else:
                                nc.tensor.transpose(
                                    ps[:, tt, i * P:(i + 1) * P],
                                    st["kvch"][pc][:, i, t * P:(t + 1) * P],
                                    ident)
                    dst = imgT_c if which == 0 else txtT_c
                    nc.vector.tensor_copy(out=dst[:, tp * 2:tp * 2 + 2, :],
                                          in_=ps)
                    g += 1
                    if g == 2 and inter:
                        inter.pop(0)()
            while inter:
                inter.pop(0)()
            st["imgT"][pc] = imgT_c
            st["txtT"][pc] = txtT_c

        def attn_half(st, s, pc, m):
            if st["attn_ps"] is None:
                st["attn_ps"] = [
                    psAttn.tile([P, c], f32, tag="attn", name=f"attn{s}_{mm}")
                    for mm in range(KC)
                ]
            for t in range(TPC):
                nc.tensor.matmul(
                    st["attn_ps"][m],
                    lhsT=st["imgT"][pc][:, t, m * P:(m + 1) * P],
                    rhs=st["txtT"][pc][:, t, :],
                    start=(pc == 0 and t == 0),
                    stop=(pc == NCH - 1 and t == TPC - 1))
            if m == KC - 1:
                st["imgT"][pc] = st["txtT"][pc] = None

        def attn_chunk(st, s, pc):
            for m in range(KC):
                attn_half(st, s, pc, m)

        def softmax(st, s, cover=()):
            # softmax over the free (d) axis, gamma folded in; transpose to
            # attnT [d, c] for the attn@kv contraction.  `cover` closures are
            # emitted between the DVE/ACT stats and the PE transposes so the
            # tensor engine has work while the serial softmax chain runs.
            attnT_sb = attnsb.tile([P, KC, c], f16, tag="attnT", name="attnT")
            st["attnT"] = attnT_sb
            exps = []
            for m in range(KC):
                nmax = smalls.tile([P, 1], f32, tag="nmax", name="nmax")
                nc.vector.reduce_max(out=nmax, in_=st["attn_ps"][m], axis=X,
                                     negate=True)
                exp_sb = smalls.tile([P, c], f16, tag="exp", name="exp_sb")
                rsum = smalls.tile([P, 1], f32, tag="rsum", name="rsum")
                nc.scalar.activation(out=exp_sb, in_=st["attn_ps"][m], func=Exp,
                                     bias=nmax, scale=1.0, accum_out=rsum)
                rg = smalls.tile([P, 1], f32, tag="rg", name="rg")
                nc.vector.reciprocal(out=rg, in_=rsum)
                nc.vector.tensor_mul(out=rg, in0=rg, in1=gamma_sb)
                nc.vector.tensor_scalar_mul(out=exp_sb, in0=exp_sb, scalar1=rg)
                exps.append(exp_sb)
            for fn in cover:
                fn()
            for m in range(KC):
                for j in range(KC):
                    pst = psB.tile([P, P], f16, tag="b", name="ps_tr")
                    nc.tensor.transpose(pst, exps[m][:, j * P:(j + 1) * P], ident)
                    nc.vector.tensor_copy(out=attnT_sb[:, j, m * P:(m + 1) * P],
                                          in_=pst)

        def ph4_chunk(st, s, cc):
            # y = gamma*attn@kv + img, overwriting img in place
            cs = cc * ch
            for m in range(KC):
                ps = psA.tile([P, ch], f32, tag="a", name="ps_ai")
                for j in range(KC):
                    nc.tensor.matmul(ps, lhsT=st["attnT"][:, j, m * P:(m + 1) * P],
                                     rhs=st["kvch"][cc][:, j, :],
                                     start=(j == 0), stop=(j == KC - 1))
                nc.vector.tensor_add(out=st["img"][:, m, cs:cs + ch], in0=ps,
                                     in1=st["img"][:, m, cs:cs + ch])
            st["kvch"][cc] = None

        def ph5_chunk(st, s, cc):
            # all four m-tiles share one fp16 staging tile and go out with
            # a single contiguous DMA on the Pool engine's software DGE,
            # keeping the HWDGE rings free for input loads.
            cs = cc * ch
            last = (s == spc - 1 and cc == NCH - 1)
            ot = ostage.tile([P, MO, ch], f16, tag="ot", name="ot")
            for m2 in range(MO):
                ps = psA.tile([P, ch], f32, tag="a", name="ps_out")
                for j in range(KC):
                    nc.tensor.matmul(ps,
                                     lhsT=woT_sb[:, j, m2 * P:(m2 + 1) * P],
                                     rhs=st["img"][:, j, cs:cs + ch],
                                     start=(j == 0), stop=(j == KC - 1))
                if m2 % 2 == 0:
                    nc.vector.tensor_copy(out=ot[:, m2, :], in_=ps)
                else:
                    nc.scalar.copy(out=ot[:, m2, :], in_=ps)
                if last and m2 == 1:
                    # stream the final chunk out in pieces across both HWDGE
                    # rings so the kernel end never waits on one big transfer
                    nc.sync.dma_start(out=out[s, cc, :, 0:2, :],
                                      in_=ot[:, 0:2, :])
                if last and m2 == 2:
                    nc.scalar.dma_start(out=out[s, cc, :, 2:3, :],
                                        in_=ot[:, 2:3, :])
            if last:
                nc.sync.dma_start(out=out[s, cc, :, 3:4, :], in_=ot[:, 3:4, :])
            elif s == spc - 1 and cc == NCH - 2:
                # second-to-last store also avoids the backlogged SWDGE ring
                nc.sync.dma_start(out=out[s, cc], in_=ot[:, :, :])
            else:
                nc.gpsimd.dma_start(out=out[s, cc], in_=ot[:, :, :])

        # ---- pipelined schedule: sample s-1's tail (last transposes, attn,
        # softmax, phases 4/5) is interleaved into sample s's pass-A chunks
        # so the PE never drains at sample boundaries.  ph4/ph5 chunks are
        # interleaved pairwise so ph4's DVE residual adds run under ph5's
        # PE-heavy output conv.
        def tail_units(st, s):
            units = []
            order = []
            order.append(("ph4", 0))
            order.append(("ph4", 1))
            for cc in range(2, NCH):
                order.append(("ph5", cc - 2))
                order.append(("ph4", cc))
            order.append(("ph5", NCH - 2))
            order.append(("ph5", NCH - 1))
            for kind, cc in order:
                if kind == "ph4":
                    units.append(lambda st=st, s=s, cc=cc: ph4_chunk(st, s, cc))
                else:
                    units.append(lambda st=st, s=s, cc=cc: ph5_chunk(st, s, cc))
            return units

        emit_warmup()
        tails = []
        for s in range(spc):
            st = {"img": None, "kvch": [None] * NCH, "attn_ps": None,
                  "attnT": None, "imgT": [None] * NCH, "txtT": [None] * NCH}
            st["img"] = imgp.tile([P, KC, hw], f16, tag="img", name=f"img{s}")
            for cc in range(NCH):
                passA_chunk(st, s, cc, first=(s == 0 and cc == 0))
                # constants stay off the head: the fabric runs ~320GB/s
                # combined over both HWDGE rings, so every non-critical byte
                # issued early directly delays the first-chunk loads
                if s == 0 and cc == 1:
                    emit_ident()
                if s == 0 and cc == 2:
                    emit_deferred_constants()
                if cc >= 1:
                    inter = ()
                    if cc >= 2:
                        inter = (lambda st=st, s=s, cc=cc: attn_half(st, s, cc - 2, 0),
                                 lambda st=st, s=s, cc=cc: attn_half(st, s, cc - 2, 1))
                    transposes(st, s, cc - 1, inter)
                # the last sample keeps one extra ph5 unit in reserve so its
                # softmax (which has no later pass-A to hide behind) gets
                # ~3.5us of covering PE work instead of ~1.7us
                if s == spc - 1:
                    npop = (3, 3, 3, 3, 2, 2, 1, 0)[min(cc, 7)]
                else:
                    npop = (3, 3, 3, 3, 2, 2, 1, 1)[min(cc, 7)]
                for _ in range(npop):
                    if tails:
                        tails.pop(0)()
            if s == spc - 1:
                # the final sample's softmax has no later pass-A to hide
                # behind; cover it with whatever of the previous sample's
                # tail is still pending (its last output-conv chunks).
                leftovers = tails[:]
                tails.clear()
                tails.extend([
                    (lambda st=st, s=s: transposes(
                        st, s, NCH - 1,
                        (lambda: attn_half(st, s, NCH - 2, 0),
                         lambda: attn_half(st, s, NCH - 2, 1)))),
                    (lambda st=st, s=s: attn_chunk(st, s, NCH - 1)),
                    (lambda st=st, s=s, cov=tuple(leftovers):
                        softmax(st, s, cover=cov)),
                ])
            else:
                tails.extend([
                    (lambda st=st, s=s: transposes(
                        st, s, NCH - 1,
                        (lambda: attn_half(st, s, NCH - 2, 0),
                         lambda: attn_half(st, s, NCH - 2, 1)))),
                    (lambda st=st, s=s: attn_chunk(st, s, NCH - 1)),
                    (lambda st=st, s=s: softmax(st, s)),
                ])
            tails.extend(tail_units(st, s))
        while tails:
            tails.pop(0)()

    nc.compile()
    return nc


def _get_nc():
    key = "full"
    if key not in _BUILD_CACHE:
        _BUILD_CACHE[key] = _build_nc()
    return _BUILD_CACHE[key]


LAST_RESULTS = None  # BassKernelResults of the most recent kernel() call

P = 128
K1 = C1 // P
K2 = C2P // P
KC = C // P
MO = C1 // P
CH = 512
NCH = HW // CH


def _tile_input(x, kt):
    """[B, kt*128, HW] -> [B, NCH, 128, kt, CH] contiguous fp16."""
    b = x.shape[0]
    t = x.reshape(b, kt, P, NCH, CH).transpose(0, 3, 2, 1, 4)
    return np.ascontiguousarray(t)


def _tile_weightT(wT, kt, cols):
    """[kt*128, cols] -> [128, kt, cols] contiguous fp16."""
    return np.ascontiguousarray(
        wT.reshape(kt, P, cols).transpose(1, 0, 2).astype(np.float16))


def kernel(x1, x2, w_img, w_txt, w_out, gamma):
    import os
    from concourse.bass_utils import run_bass_kernel_spmd

    x1 = np.asarray(x1, dtype=np.float32).reshape(B, C1, HW).astype(np.float16)
    x2 = np.asarray(x2, dtype=np.float32).reshape(B, C2, HW).astype(np.float16)
    w_img = np.asarray(w_img, dtype=np.float32)
    w_txt = np.asarray(w_txt, dtype=np.float32)
    w_out = np.asarray(w_out, dtype=np.float32)
    gamma = np.ascontiguousarray(np.asarray(gamma, dtype=np.float32)).reshape(1)

    # pad x2 channels 320 -> 384 with zeros so every k-tile is 128 deep
    x2p = np.zeros((B, C2P, HW), dtype=np.float16)
    x2p[:, :C2, :] = x2

    x1t = _tile_input(x1, K1)                     # [B, NCH, P, K1, CH]
    x2t = _tile_input(x2p, K2)                    # [B, NCH, P, K2, CH]

    w_imgT = _tile_weightT(w_img.T, K1, C)        # [P, K1, 256]
    w_txtTf = np.zeros((C2P, C), dtype=np.float16)
    w_txtTf[:C2, :] = w_txt.T.astype(np.float16)
    w_txtT = _tile_weightT(w_txtTf, K2, C)        # [P, K2, 256]
    w_outT = _tile_weightT(w_out.T, KC, C1)       # [P, KC, 512]

    nc = _get_nc()
    ident = np.eye(128, dtype=np.float16)
    in_maps = []
    for core in range(NCORES):
        s0 = core * SPC
        in_maps.append({
            "x1": np.ascontiguousarray(x1t[s0:s0 + SPC]),
            "x2": np.ascontiguousarray(x2t[s0:s0 + SPC]),
            "w_imgT": w_imgT,
            "w_txtT": w_txtT,
            "w_outT": w_outT,
            "gamma": gamma,
            "ident": ident,
            "nonce": np.zeros((1, _nonce_len()), dtype=np.float32),
        })

    kwargs = {}
    if os.environ.get("KERNEL_TRACE"):
        kwargs["trace"] = True
        if os.environ.get("KERNEL_TRACE_DIR"):
            kwargs["tmpdir"] = os.environ["KERNEL_TRACE_DIR"]
    res = run_bass_kernel_spmd(nc, in_maps, core_ids=list(range(NCORES)), **kwargs)
    global LAST_RESULTS
    LAST_RESULTS = res
    outs = [res.results[c]["out"] for c in range(NCORES)]
    full_t = np.concatenate(outs, axis=0)          # [B, NCH, P, MO, CH] f16
    # un-tile: out[s, m*128+p, cc*512+j] = full_t[s, cc, p, m, j]
    full = full_t.transpose(0, 3, 2, 1, 4).reshape(B, C1, HW)
    return full.astype(np.float32).reshape(B, C1, 64, 64)


if __name__ == "__main__":
    rng = np.random.default_rng(0)
    inputs = {
        "x1": rng.standard_normal((B, C1, 64, 64), dtype=np.float32),
        "x2": rng.standard_normal((B, C2, 64, 64), dtype=np.float32),
        "w_img": rng.standard_normal((C, C1), dtype=np.float32) / np.sqrt(C1),
        "w_txt": rng.standard_normal((C, C2), dtype=np.float32) / np.sqrt(C2),
        "w_out": rng.standard_normal((C1, C), dtype=np.float32) / np.sqrt(C),
        "gamma": rng.standard_normal(1).astype(np.float32),
    }
    out = kernel(**inputs)
    print(out.shape, out.dtype)
